# revision 13
# baseline (speedup 1.0000x reference)
"""Trainium2 Bass kernel for MemVim (memory-bank attention + gated fusion + decoder).

Sharding: data-parallel over batch for the attention/gating stages (4 samples
per core), tensor-parallel (row-sharded dec_w) for the 1024x25088 decoder
linear with an AllToAll on pooled features and a ReduceScatter on the decoder
output, then data-parallel conv-transpose decode of each core's own samples.
"""
import sys, os, types

sys.path.insert(0, "/opt/trn_rl_repo")


def _install_ntff_hook():
    try:
        from antenv.axon_hooks import get_axon_ntff_profile_hook  # noqa
        return
    except ImportError:
        pass
    try:
        from trn_agent_boot.trn_boot import _ntff_profile_via_ctypes
        hook = _ntff_profile_via_ctypes("/opt/axon/libaxon_pjrt.so")
    except Exception:
        hook = None
    mod = types.ModuleType("antenv.axon_hooks")
    mod.get_axon_ntff_profile_hook = lambda: hook
    mod.set_axon_ntff_profile_hook = lambda h: None
    sys.modules["antenv.axon_hooks"] = mod


_install_ntff_hook()

import numpy as np
from contextlib import ExitStack
import concourse.bass as bass
import concourse.mybir as mybir
import concourse.tile as tile
from concourse.bass_utils import run_bass_kernel_spmd
from concourse.masks import make_identity
from concourse.vector_clock import ScopedClock, VectorClock


# The walrus build in this image rejects InstDrain carrying fused sync
# commands and instructions with >1 sync wait. Replace the stock barriers
# with sem-only (sequencer-level) barriers and spread the tail drain's
# global-clock waits across single-wait NOPs. The waits still prove every
# DMA/collective semaphore increment landed before the sem reset.
def _patched_meb(self, engines):
    for inst in self._sem_only_all_engine_barrier_insts("aeb"):
        self.engines[inst.engine].add_instruction(inst)


def _patched_dab(self, tick_clock, wait_clock):
    gc = tick_clock.global_clock
    n = len(gc)
    procs = [i for i in range(n) if gc[i] > 0]
    for j, p in enumerate(procs):
        nop_inst = self.nc.sync.nop(nofuse=True, hint=f"tailwait{j}")
        vec = [0] * n
        vec[p] = gc[p]
        wait_clock.add_sem_waits(nop_inst.ins, ScopedClock({None: VectorClock(vec)}))
    self.nc.sync.drain()
    self.nc.all_engine_barrier(sem_only=True)
    popped = self.nc._tile_sem_poison_stack.pop()
    assert popped is self._sem_poison
    self.nc.clear_and_free_semaphores(list(self.sems.allocated().values()))
    self.nc.all_engine_barrier(sem_only=True)


bass.Bass.multi_engine_barrier = _patched_meb
tile.TileContext._drain_and_barrier = _patched_dab

# No remote artifact bucket in this container; keep the profile local.
import concourse.bass_utils as _bu
_bu.upload_artifacts = lambda tmpdir: str(tmpdir)


def _legalize_sync(nc):
    """This walrus build encodes at most one sync wait and one sync update
    per instruction, and none at all on InstDrain. Move extra waits onto
    NOPs inserted just before the instruction (same engine => same stream
    order) and extra updates onto NOPs just after."""
    ctr = [0]

    def nop(engine, waits, updates):
        ctr[0] += 1
        n = mybir.InstNoOp(name=f"lsw-{ctr[0]}", ins=[], outs=[])
        n.engine = engine
        n.sync_info = mybir.SyncInfo(on_wait=list(waits), on_update=list(updates))
        return n

    for fn in nc.m.functions:
        for bb in fn.blocks:
            out = []
            for ins in bb.instructions:
                si = ins.sync_info
                if si is None:
                    out.append(ins)
                    continue
                waits = list(si.on_wait)
                updates = list(si.on_update)
                is_drain = type(ins).__name__ == "InstDrain"
                kw = 0 if is_drain else 1
                ku = 0 if is_drain else 1
                extra_w = waits[kw:]
                extra_u = updates[ku:]
                if not extra_w and not extra_u:
                    out.append(ins)
                    continue
                for w in extra_w:
                    out.append(nop(ins.engine, [w], []))
                ins.sync_info = mybir.SyncInfo(on_wait=waits[:kw], on_update=updates[:ku])
                out.append(ins)
                for u in extra_u:
                    out.append(nop(ins.engine, [], [u]))
            bb.instructions = out

fp32 = mybir.dt.float32
AF = mybir.ActivationFunctionType
ALU = mybir.AluOpType
AX = mybir.AxisListType

NCORES = 8
P = 128
B, N, D, K = 32, 196, 1024, 512
BL = B // NCORES          # 4 samples per core
T = BL * N                # 784 tokens per core
TT = 7                    # token tiles (6 full + 1 of 16)
DT = D // P               # 8
PT = K // P               # 4
DEC_N = 7 * 7 * 512       # 25088
DEC_CH = 7                # dec column chunks
DEC_CW = DEC_N // DEC_CH  # 3584
RG = [list(range(NCORES))]

# taps[parity] = list of (pad_offset_delta, kernel_index) for k=4,s=2,p=1
TAPS = {0: [(1, 1), (0, 3)], 1: [(2, 0), (1, 2)]}


def _build():
    nc = bass.Bass()
    z_d = nc.dram_tensor("z", [T, D], fp32, kind="ExternalInput")
    p_d = nc.dram_tensor("prototypes", [K, D], fp32, kind="ExternalInput")
    gw_d = nc.dram_tensor("gate_w", [2 * D, D], fp32, kind="ExternalInput")
    gb_d = nc.dram_tensor("gate_b", [D], fp32, kind="ExternalInput")
    dw_d = nc.dram_tensor("dec_w_shard", [P, DEC_N], fp32, kind="ExternalInput")
    db_d = nc.dram_tensor("dec_b", [DEC_N], fp32, kind="ExternalInput")
    w1_d = nc.dram_tensor("ct1_w", [512, 256, 4, 4], fp32, kind="ExternalInput")
    b1_d = nc.dram_tensor("ct1_b", [256], fp32, kind="ExternalInput")
    w2_d = nc.dram_tensor("ct2_w", [256, 3, 4, 4], fp32, kind="ExternalInput")
    b2_d = nc.dram_tensor("ct2_b", [3], fp32, kind="ExternalInput")
    attn_o = nc.dram_tensor("attn_out", [T, K], fp32, kind="ExternalOutput")
    xr_o = nc.dram_tensor("xr_out", [BL, 3, 28, 28], fp32, kind="ExternalOutput")

    with tile.TileContext(nc) as tc, ExitStack() as ctx:
        const = ctx.enter_context(tc.tile_pool(name="const", bufs=1))
        dram = ctx.enter_context(tc.tile_pool(name="dram", bufs=1, space="DRAM"))
        ps = ctx.enter_context(tc.tile_pool(name="ps", bufs=6, space="PSUM"))
        work = ctx.enter_context(tc.tile_pool(name="work", bufs=2))

        def psum(name):
            return ps.tile([P, 512], fp32, tag="ps", name=name)

        ident = const.tile([P, P], fp32, name="ident")
        make_identity(nc, ident[:])
        eps_sb = const.tile([P, 1], fp32, name="eps_sb")
        nc.gpsimd.memset(eps_sb[:], 1e-30)
        gb_sb = const.tile([P, DT], fp32, name="gb_sb")
        nc.sync.dma_start(gb_sb[:], gb_d[:].rearrange("(o p) -> p o", p=P))
        b1_sb = const.tile([P, 2], fp32, name="b1_sb")
        nc.sync.dma_start(b1_sb[:], b1_d[:].rearrange("(o p) -> p o", p=P))
        b2_sb = const.tile([3, 1], fp32, name="b2_sb")
        nc.sync.dma_start(b2_sb[:], b2_d[:][:, None])

        with ExitStack() as stageA:
            poolZ = stageA.enter_context(tc.tile_pool(name="poolZ", bufs=1))
            stagePA = ExitStack()
            poolP = stagePA.enter_context(tc.tile_pool(name="poolP", bufs=1))
            poolAt = stagePA.enter_context(tc.tile_pool(name="poolAt", bufs=1))

            # ---- Phase 1: prototypes: load, row-normalize, transpose ----
            p_nat = [poolP.tile([P, D], fp32, name=f"p_nat{i}") for i in range(PT)]
            ptn = [poolP.tile([P, K], fp32, name=f"ptn{i}") for i in range(DT)]
            pn2 = const.tile([P, PT], fp32, name="pn2")
            pninv = const.tile([P, PT], fp32, name="pninv")
            for pt in range(PT):
                nc.sync.dma_start(p_nat[pt][:], p_d[pt * P:(pt + 1) * P, :])
                sq = work.tile([P, D], fp32, name="sq", tag="sq")
                nc.scalar.activation(sq[:], p_nat[pt][:], AF.Square,
                                     accum_out=pn2[:, pt:pt + 1])
            nc.scalar.activation(pninv[:], pn2[:], AF.Sqrt, bias=eps_sb[:])
            nc.vector.reciprocal(pninv[:], pninv[:])
            for pt in range(PT):
                pnrm = work.tile([P, D], fp32, name="pnrm", tag="pnrm")
                nc.scalar.activation(pnrm[:], p_nat[pt][:], AF.Copy,
                                     scale=pninv[:, pt:pt + 1])
                for dt in range(DT):
                    ptp = psum(f"ptp{pt}_{dt}")
                    nc.tensor.transpose(ptp[:, :P], pnrm[:, dt * P:(dt + 1) * P], ident[:])
                    nc.vector.tensor_copy(ptn[dt][:, pt * P:(pt + 1) * P], ptp[:, :P])

            # ---- Phase 2: z: load, norms, transpose ----
            zT = [poolZ.tile([P, TT * P], fp32, name=f"zT{i}") for i in range(DT)]
            zn2 = const.tile([P, TT], fp32, name="zn2")
            zninv = const.tile([P, TT], fp32, name="zninv")
            for tt in range(TT):
                z_t = work.tile([P, D], fp32, name="z_t", tag="z_t")
                rows = P if tt < 6 else T - 6 * P
                if rows < P:
                    nc.gpsimd.memset(z_t[:], 0.0)
                nc.sync.dma_start(z_t[:rows, :], z_d[tt * P: tt * P + rows, :])
                sq = work.tile([P, D], fp32, name="sqz", tag="sq")
                nc.scalar.activation(sq[:], z_t[:], AF.Square,
                                     accum_out=zn2[:, tt:tt + 1])
                for dt in range(DT):
                    ztp = psum(f"ztp{tt}_{dt}")
                    nc.tensor.transpose(ztp[:, :P], z_t[:, dt * P:(dt + 1) * P], ident[:])
                    nc.vector.tensor_copy(zT[dt][:, tt * P:(tt + 1) * P], ztp[:, :P])
            nc.scalar.activation(zninv[:], zn2[:], AF.Sqrt, bias=eps_sb[:])
            nc.vector.reciprocal(zninv[:], zninv[:])

            # ---- Phase 3: dots, softmax, attn out, attn transpose ----
            attnT = [poolAt.tile([P, TT * P], fp32, name=f"attnT{i}") for i in range(PT)]
            for tt in range(TT):
                ps_d = psum(f"dots{tt}")
                for dt in range(DT):
                    nc.tensor.matmul(ps_d[:, :K], lhsT=zT[dt][:, tt * P:(tt + 1) * P],
                                     rhs=ptn[dt][:], start=(dt == 0), stop=(dt == DT - 1))
                at = poolAt.tile([P, K], fp32, name=f"attn{tt}")
                nc.scalar.activation(at[:], ps_d[:, :K], AF.Copy,
                                     scale=zninv[:, tt:tt + 1])
                nmax = work.tile([P, 1], fp32, name="nmax", tag="nmax")
                nc.vector.tensor_reduce(out=nmax[:], in_=at[:], axis=AX.X,
                                        op=ALU.max, negate=True)
                esum = work.tile([P, 1], fp32, name="esum", tag="esum")
                nc.scalar.activation(at[:], at[:], AF.Exp, bias=nmax[:],
                                     accum_out=esum[:])
                nc.vector.reciprocal(esum[:], esum[:])
                nc.scalar.activation(at[:], at[:], AF.Copy, scale=esum[:])
                rows = P if tt < 6 else T - 6 * P
                nc.sync.dma_start(attn_o[tt * P: tt * P + rows, :], at[:rows, :])
                for pt in range(PT):
                    atp = psum(f"atp{tt}_{pt}")
                    nc.tensor.transpose(atp[:, :P], at[:, pt * P:(pt + 1) * P], ident[:])
                    nc.vector.tensor_copy(attnT[pt][:, tt * P:(tt + 1) * P], atp[:, :P])

            # ---- Phase 4: z_hatT = (attn @ P)^T ----
            NSP = TT * P // 2  # 448
            zhT = [poolZ.tile([P, TT * P], fp32, name=f"zhT{i}") for i in range(DT)]
            for dt in range(DT):
                for n2 in range(2):
                    sl = slice(n2 * NSP, (n2 + 1) * NSP)
                    ps_z = psum(f"zh{dt}_{n2}")
                    for pt in range(PT):
                        nc.tensor.matmul(ps_z[:, :NSP],
                                         lhsT=p_nat[pt][:, dt * P:(dt + 1) * P],
                                         rhs=attnT[pt][:, sl],
                                         start=(pt == 0), stop=(pt == PT - 1))
                    nc.vector.tensor_copy(zhT[dt][:, sl], ps_z[:, :NSP])
            # ---- Phase 5: gate, fused (in-place into zhT), pooled ----
            stagePA.close()  # free prototype/attn pools before gate weights
            poolG = stageA.enter_context(tc.tile_pool(name="poolG", bufs=1))
            gw_sb = [poolG.tile([P, D], fp32, name=f"gw{i}") for i in range(16)]
            for kt in range(16):
                nc.sync.dma_start(gw_sb[kt][:], gw_d[kt * P:(kt + 1) * P, :])
            g_all = [poolZ.tile([P, TT * P], fp32, name=f"g_all{i}") for i in range(DT)]
            pooledT = const.tile([P, DT, BL], fp32, name="pooledT")
            # 5a: all gates first (zhT must stay pristine while it feeds rhs)
            for dt in range(DT):
                for n2 in range(2):
                    sl = slice(n2 * NSP, (n2 + 1) * NSP)
                    ps_g = psum(f"g{dt}_{n2}")
                    for kt in range(16):
                        rhs = zT[kt][:, sl] if kt < 8 else zhT[kt - 8][:, sl]
                        nc.tensor.matmul(ps_g[:, :NSP],
                                         lhsT=gw_sb[kt][:, dt * P:(dt + 1) * P],
                                         rhs=rhs, start=(kt == 0), stop=(kt == 15))
                    nc.scalar.activation(g_all[dt][:, sl], ps_g[:, :NSP], AF.Sigmoid,
                                         bias=gb_sb[:, dt:dt + 1])
            # 5b: fused (in-place into zhT) + pooling
            for dt in range(DT):
                for n2 in range(2):
                    sl = slice(n2 * NSP, (n2 + 1) * NSP)
                    dif = work.tile([P, NSP], fp32, name="dif", tag="dif")
                    nc.vector.tensor_sub(dif[:], zT[dt][:, sl], zhT[dt][:, sl])
                    nc.vector.tensor_mul(dif[:], g_all[dt][:, sl], dif[:])
                    nc.vector.tensor_add(zhT[dt][:, sl], zhT[dt][:, sl], dif[:])
                for s in range(BL):
                    nc.vector.reduce_sum(out=pooledT[:, dt, s:s + 1],
                                         in_=zhT[dt][:, s * N:(s + 1) * N], axis=AX.X)
            nc.vector.tensor_scalar_mul(pooledT[:], pooledT[:], 1.0 / N)

        # ---- Phase 6: AllToAll pooled ----
        a2a_in = dram.tile([D, BL], fp32, name="a2a_in")
        a2a_out = dram.tile([D, BL], fp32, name="a2a_out")
        nc.sync.dma_start(a2a_in[:].rearrange("(o p) s -> p o s", p=P), pooledT[:])
        nc.gpsimd.collective_compute("AllToAll", ALU.bypass, replica_groups=RG,
                                     ins=[a2a_in.opt()], outs=[a2a_out.opt()])
        pf = const.tile([P, NCORES, BL], fp32, name="pf")
        nc.sync.dma_start(pf[:], a2a_out[:].rearrange("(c p) s -> p c s", p=P))
        pf2 = pf[:].rearrange("p c s -> p (c s)")

        # ---- Phase 7: dec matmul (partial, my 128 input rows) + RS ----
        rs_in = dram.tile([B, DEC_N], fp32, name="rs_in")
        rs_out = dram.tile([BL, DEC_N], fp32, name="rs_out")
        with ExitStack() as stageD:
            poolD = stageD.enter_context(tc.tile_pool(name="poolD", bufs=3))
            for ch in range(DEC_CH):
                dw_t = poolD.tile([P, DEC_CW], fp32, name="dw_t", tag="dw_t")
                nc.sync.dma_start(dw_t[:], dw_d[:, ch * DEC_CW:(ch + 1) * DEC_CW])
                h_t = poolD.tile([B, DEC_CW], fp32, name="h_t", tag="h_t")
                for nn in range(DEC_CW // 512):
                    ps_h = psum(f"h{ch}_{nn}")
                    nc.tensor.matmul(ps_h[:B, :], lhsT=pf2,
                                     rhs=dw_t[:, nn * 512:(nn + 1) * 512],
                                     start=True, stop=True)
                    nc.vector.tensor_copy(h_t[:, nn * 512:(nn + 1) * 512], ps_h[:B, :])
                nc.sync.dma_start(rs_in[:, ch * DEC_CW:(ch + 1) * DEC_CW], h_t[:])
        nc.gpsimd.collective_compute("ReduceScatter", ALU.add, replica_groups=RG,
                                     ins=[rs_in.opt()], outs=[rs_out.opt()])

        # ---- Phase 8: convT1 (512->256, 7x7 -> 14x14) ----
        with ExitStack() as stageC:
            poolC = stageC.enter_context(tc.tile_pool(name="poolC", bufs=1))
            h4 = rs_out[:].rearrange("s (c h w) -> c s h w", c=512, h=7)
            xpad = [poolC.tile([P, BL, 9, 9], fp32, name=f"xpad{i}") for i in range(4)]
            w1 = [poolC.tile([P, 256, 4, 4], fp32, name=f"w1_{i}") for i in range(4)]
            db4 = db_d[:].rearrange("(c h w) -> c h w", c=512, h=7)
            for ci in range(4):
                nc.gpsimd.memset(xpad[ci][:], 0.0)
                for s in range(BL):
                    nc.sync.dma_start(xpad[ci][:, s, 1:8, 1:8],
                                      h4[ci * P:(ci + 1) * P, s])
                bd = work.tile([P, 7, 7], fp32, name="bd", tag="bd")
                nc.sync.dma_start(bd[:], db4[ci * P:(ci + 1) * P])
                nc.vector.tensor_add(xpad[ci][:, :, 1:8, 1:8], xpad[ci][:, :, 1:8, 1:8],
                                     bd[:][:, None].to_broadcast([P, BL, 7, 7]))
                nc.sync.dma_start(w1[ci][:], w1_d[ci * P:(ci + 1) * P])
            y1pad = [poolC.tile([P, BL, 16, 16], fp32, name=f"y1pad{i}") for i in range(2)]
            for mo in range(2):
                nc.gpsimd.memset(y1pad[mo][:], 0.0)
            for po in range(2):
                for pw in range(2):
                    for mo in range(2):
                        ps_c = psum(f"c1_{po}{pw}{mo}")
                        pc4 = ps_c[:, :196].rearrange("p (s h w) -> p s h w", s=BL, h=7)
                        idx = 0
                        for ci in range(4):
                            for (dh, kh) in TAPS[po]:
                                for (dw_, kw) in TAPS[pw]:
                                    nc.tensor.matmul(
                                        pc4,
                                        lhsT=w1[ci][:, mo * P:(mo + 1) * P, kh, kw],
                                        rhs=xpad[ci][:, :, dh:dh + 7, dw_:dw_ + 7],
                                        start=(idx == 0), stop=(idx == 15))
                                    idx += 1
                        nc.scalar.activation(
                            y1pad[mo][:, :, 1 + po:15:2, 1 + pw:15:2], pc4,
                            AF.Relu, bias=b1_sb[:, mo:mo + 1])

            # ---- Phase 9: convT2 (256->3, 14x14 -> 28x28) ----
            w2 = [poolC.tile([P, 3, 4, 4], fp32, name=f"w2_{i}") for i in range(2)]
            for ci in range(2):
                nc.sync.dma_start(w2[ci][:], w2_d[ci * P:(ci + 1) * P])
            xr = poolC.tile([3, BL, 28, 28], fp32, name="xr")
            for po in range(2):
                for pw in range(2):
                    for half in range(2):
                        ps_x = psum(f"c2_{po}{pw}{half}")
                        px4 = ps_x[:3, :392].rearrange("p (s h w) -> p s h w", s=BL, h=14)
                        idx = 0
                        for ci in range(2):
                            for (dh, kh) in TAPS[po]:
                                for (dw_, kw) in TAPS[pw]:
                                    nc.tensor.matmul(
                                        px4,
                                        lhsT=w2[ci][:, :, kh, kw],
                                        rhs=y1pad[ci][:, :, dh:dh + 14,
                                                      dw_ + 7 * half:dw_ + 7 * half + 7],
                                        start=(idx == 0), stop=(idx == 7))
                                    idx += 1
                        nc.scalar.activation(
                            xr[:, :, po:28:2, pw + 14 * half:pw + 14 * half + 13:2],
                            px4, AF.Identity, bias=b2_sb[:])
            nc.sync.dma_start(xr_o[:].rearrange("s c h w -> c s h w"), xr[:])
    _legalize_sync(nc)
    return nc


_NC_CACHE = None


def _get_nc():
    global _NC_CACHE
    if _NC_CACHE is None:
        _NC_CACHE = _build()
    return _NC_CACHE


def _run(inputs, trace=False):
    nc = _get_nc()
    z = np.ascontiguousarray(np.asarray(inputs["z"], dtype=np.float32))
    dec_w = np.asarray(inputs["dec_w"], dtype=np.float32)
    shared = {
        "prototypes": np.ascontiguousarray(inputs["prototypes"], dtype=np.float32),
        "gate_w": np.ascontiguousarray(inputs["gate_w"], dtype=np.float32),
        "gate_b": np.ascontiguousarray(inputs["gate_b"], dtype=np.float32),
        "dec_b": np.ascontiguousarray(inputs["dec_b"], dtype=np.float32),
        "ct1_w": np.ascontiguousarray(inputs["ct1_w"], dtype=np.float32),
        "ct1_b": np.ascontiguousarray(inputs["ct1_b"], dtype=np.float32),
        "ct2_w": np.ascontiguousarray(inputs["ct2_w"], dtype=np.float32),
        "ct2_b": np.ascontiguousarray(inputs["ct2_b"], dtype=np.float32),
    }
    in_maps = []
    for c in range(NCORES):
        m = dict(shared)
        m["z"] = np.ascontiguousarray(z[c * BL:(c + 1) * BL].reshape(T, D))
        m["dec_w_shard"] = np.ascontiguousarray(dec_w[c * P:(c + 1) * P])
        in_maps.append(m)
    res = run_bass_kernel_spmd(nc, in_maps, list(range(NCORES)), trace=trace)
    x_recon = np.concatenate([res.results[c]["xr_out"] for c in range(NCORES)], axis=0)
    attn = np.concatenate(
        [res.results[c]["attn_out"].reshape(BL, N, K) for c in range(NCORES)], axis=0)
    return (x_recon, attn), res.exec_time_ns


def kernel(**inputs):
    out, _ = _run(inputs, trace=False)
    return out


if __name__ == "__main__":
    rng = np.random.default_rng(0)
    ins = dict(
        z=rng.standard_normal((B, N, D), dtype=np.float32),
        prototypes=rng.standard_normal((K, D), dtype=np.float32),
        gate_w=(rng.standard_normal((2 * D, D), dtype=np.float32) * 0.02),
        gate_b=np.zeros(D, np.float32),
        dec_w=(rng.standard_normal((D, DEC_N), dtype=np.float32) * 0.02),
        dec_b=np.zeros(DEC_N, np.float32),
        ct1_w=(rng.standard_normal((512, 256, 4, 4), dtype=np.float32) * 0.02),
        ct1_b=np.zeros(256, np.float32),
        ct2_w=(rng.standard_normal((256, 3, 4, 4), dtype=np.float32) * 0.02),
        ct2_b=np.zeros(3, np.float32),
    )
    out, t = _run(ins, trace=False)
    print("shapes:", out[0].shape, out[1].shape, "exec_ns:", t)


# revision 14
# speedup vs baseline: 1.0171x; 1.0171x over previous
"""Trainium2 Bass kernel for MemVim (memory-bank attention + gated fusion + decoder).

Sharding: data-parallel over batch for the attention/gating stages (4 samples
per core), tensor-parallel (row-sharded dec_w) for the 1024x25088 decoder
linear with an AllToAll on pooled features and a ReduceScatter on the decoder
output, then data-parallel conv-transpose decode of each core's own samples.
"""
import sys, os, types

sys.path.insert(0, "/opt/trn_rl_repo")


def _install_ntff_hook():
    try:
        from antenv.axon_hooks import get_axon_ntff_profile_hook  # noqa
        return
    except ImportError:
        pass
    try:
        from trn_agent_boot.trn_boot import _ntff_profile_via_ctypes
        hook = _ntff_profile_via_ctypes("/opt/axon/libaxon_pjrt.so")
    except Exception:
        hook = None
    mod = types.ModuleType("antenv.axon_hooks")
    mod.get_axon_ntff_profile_hook = lambda: hook
    mod.set_axon_ntff_profile_hook = lambda h: None
    sys.modules["antenv.axon_hooks"] = mod


_install_ntff_hook()

import numpy as np
from contextlib import ExitStack
import concourse.bass as bass
import concourse.mybir as mybir
import concourse.tile as tile
from concourse.bass_utils import run_bass_kernel_spmd
from concourse.masks import make_identity
from concourse.vector_clock import ScopedClock, VectorClock


# The walrus build in this image rejects InstDrain carrying fused sync
# commands and instructions with >1 sync wait. Replace the stock barriers
# with sem-only (sequencer-level) barriers and spread the tail drain's
# global-clock waits across single-wait NOPs. The waits still prove every
# DMA/collective semaphore increment landed before the sem reset.
def _patched_meb(self, engines):
    for inst in self._sem_only_all_engine_barrier_insts("aeb"):
        self.engines[inst.engine].add_instruction(inst)


def _patched_dab(self, tick_clock, wait_clock):
    gc = tick_clock.global_clock
    n = len(gc)
    procs = [i for i in range(n) if gc[i] > 0]
    for j, p in enumerate(procs):
        nop_inst = self.nc.sync.nop(nofuse=True, hint=f"tailwait{j}")
        vec = [0] * n
        vec[p] = gc[p]
        wait_clock.add_sem_waits(nop_inst.ins, ScopedClock({None: VectorClock(vec)}))
    self.nc.sync.drain()
    self.nc.all_engine_barrier(sem_only=True)
    popped = self.nc._tile_sem_poison_stack.pop()
    assert popped is self._sem_poison
    self.nc.clear_and_free_semaphores(list(self.sems.allocated().values()))
    self.nc.all_engine_barrier(sem_only=True)


bass.Bass.multi_engine_barrier = _patched_meb
tile.TileContext._drain_and_barrier = _patched_dab

# No remote artifact bucket in this container; keep the profile local.
import concourse.bass_utils as _bu
_bu.upload_artifacts = lambda tmpdir: str(tmpdir)


def _legalize_sync(nc):
    """This walrus build encodes at most one sync wait and one sync update
    per instruction, and none at all on InstDrain. Move extra waits onto
    NOPs inserted just before the instruction (same engine => same stream
    order) and extra updates onto NOPs just after."""
    ctr = [0]

    def nop(engine, waits, updates):
        ctr[0] += 1
        n = mybir.InstNoOp(name=f"lsw-{ctr[0]}", ins=[], outs=[])
        n.engine = engine
        n.sync_info = mybir.SyncInfo(on_wait=list(waits), on_update=list(updates))
        return n

    for fn in nc.m.functions:
        for bb in fn.blocks:
            out = []
            for ins in bb.instructions:
                si = ins.sync_info
                if si is None:
                    out.append(ins)
                    continue
                waits = list(si.on_wait)
                updates = list(si.on_update)
                is_drain = type(ins).__name__ == "InstDrain"
                kw = 0 if is_drain else 1
                ku = 0 if is_drain else 1
                extra_w = waits[kw:]
                extra_u = updates[ku:]
                if not extra_w and not extra_u:
                    out.append(ins)
                    continue
                for w in extra_w:
                    out.append(nop(ins.engine, [w], []))
                ins.sync_info = mybir.SyncInfo(on_wait=waits[:kw], on_update=updates[:ku])
                out.append(ins)
                for u in extra_u:
                    out.append(nop(ins.engine, [], [u]))
            bb.instructions = out

fp32 = mybir.dt.float32
AF = mybir.ActivationFunctionType
ALU = mybir.AluOpType
AX = mybir.AxisListType

NCORES = 8
P = 128
B, N, D, K = 32, 196, 1024, 512
BL = B // NCORES          # 4 samples per core
T = BL * N                # 784 tokens per core
TT = 7                    # token tiles (6 full + 1 of 16)
DT = D // P               # 8
PT = K // P               # 4
DEC_N = 7 * 7 * 512       # 25088
DEC_CH = 7                # dec column chunks
DEC_CW = DEC_N // DEC_CH  # 3584
RG = [list(range(NCORES))]

# taps[parity] = list of (pad_offset_delta, kernel_index) for k=4,s=2,p=1
TAPS = {0: [(1, 1), (0, 3)], 1: [(2, 0), (1, 2)]}


def _build():
    nc = bass.Bass()
    z_d = nc.dram_tensor("z", [T, D], fp32, kind="ExternalInput")
    p_d = nc.dram_tensor("prototypes", [K, D], fp32, kind="ExternalInput")
    gw_d = nc.dram_tensor("gate_w", [2 * D, D], fp32, kind="ExternalInput")
    gb_d = nc.dram_tensor("gate_b", [D], fp32, kind="ExternalInput")
    dw_d = nc.dram_tensor("dec_w_shard", [P, DEC_N], fp32, kind="ExternalInput")
    db_d = nc.dram_tensor("dec_b", [DEC_N], fp32, kind="ExternalInput")
    w1_d = nc.dram_tensor("ct1_w", [512, 256, 4, 4], fp32, kind="ExternalInput")
    b1_d = nc.dram_tensor("ct1_b", [256], fp32, kind="ExternalInput")
    w2_d = nc.dram_tensor("ct2_w", [256, 3, 4, 4], fp32, kind="ExternalInput")
    b2_d = nc.dram_tensor("ct2_b", [3], fp32, kind="ExternalInput")
    attn_o = nc.dram_tensor("attn_out", [T, K], fp32, kind="ExternalOutput")
    xr_o = nc.dram_tensor("xr_out", [BL, 3, 28, 28], fp32, kind="ExternalOutput")

    with tile.TileContext(nc) as tc, ExitStack() as ctx:
        const = ctx.enter_context(tc.tile_pool(name="const", bufs=1))
        dram = ctx.enter_context(tc.tile_pool(name="dram", bufs=1, space="DRAM"))
        ps = ctx.enter_context(tc.tile_pool(name="ps", bufs=8, space="PSUM"))
        work = ctx.enter_context(tc.tile_pool(name="work", bufs=2))

        def psum(name):
            return ps.tile([P, 512], fp32, tag="ps", name=name)

        ident = const.tile([P, P], fp32, name="ident")
        make_identity(nc, ident[:])
        eps_sb = const.tile([P, 1], fp32, name="eps_sb")
        nc.gpsimd.memset(eps_sb[:], 1e-30)
        gb_sb = const.tile([P, DT], fp32, name="gb_sb")
        nc.sync.dma_start(gb_sb[:], gb_d[:].rearrange("(o p) -> p o", p=P))
        b1_sb = const.tile([P, 2], fp32, name="b1_sb")
        nc.sync.dma_start(b1_sb[:], b1_d[:].rearrange("(o p) -> p o", p=P))
        b2_sb = const.tile([3, 1], fp32, name="b2_sb")
        nc.sync.dma_start(b2_sb[:], b2_d[:][:, None])

        with ExitStack() as stageA:
            poolZ = stageA.enter_context(tc.tile_pool(name="poolZ", bufs=1))
            stagePA = ExitStack()
            poolP = stagePA.enter_context(tc.tile_pool(name="poolP", bufs=1))
            poolAt = stagePA.enter_context(tc.tile_pool(name="poolAt", bufs=1))

            # ---- Phase 1: prototypes: load, row-normalize, transpose ----
            p_nat = [poolP.tile([P, D], fp32, name=f"p_nat{i}") for i in range(PT)]
            ptn = [poolP.tile([P, K], fp32, name=f"ptn{i}") for i in range(DT)]
            pn2 = const.tile([P, PT], fp32, name="pn2")
            pninv = const.tile([P, PT], fp32, name="pninv")
            for pt in range(PT):
                nc.sync.dma_start(p_nat[pt][:], p_d[pt * P:(pt + 1) * P, :])
                sq = work.tile([P, D], fp32, name="sq", tag="sq")
                nc.scalar.activation(sq[:], p_nat[pt][:], AF.Square,
                                     accum_out=pn2[:, pt:pt + 1])
            nc.scalar.activation(pninv[:], pn2[:], AF.Sqrt, bias=eps_sb[:])
            nc.vector.reciprocal(pninv[:], pninv[:])
            for pt in range(PT):
                pnrm = work.tile([P, D], fp32, name="pnrm", tag="pnrm")
                nc.scalar.activation(pnrm[:], p_nat[pt][:], AF.Copy,
                                     scale=pninv[:, pt:pt + 1])
                for dt in range(DT):
                    ptp = psum(f"ptp{pt}_{dt}")
                    nc.tensor.transpose(ptp[:, :P], pnrm[:, dt * P:(dt + 1) * P], ident[:])
                    nc.vector.tensor_copy(ptn[dt][:, pt * P:(pt + 1) * P], ptp[:, :P])

            # ---- Phase 2: z: load, norms, transpose ----
            zT = [poolZ.tile([P, TT * P], fp32, name=f"zT{i}") for i in range(DT)]
            zn2 = const.tile([P, TT], fp32, name="zn2")
            zninv = const.tile([P, TT], fp32, name="zninv")
            for tt in range(TT):
                z_t = work.tile([P, D], fp32, name="z_t", tag="z_t")
                rows = P if tt < 6 else T - 6 * P
                if rows < P:
                    nc.gpsimd.memset(z_t[:], 0.0)
                nc.sync.dma_start(z_t[:rows, :], z_d[tt * P: tt * P + rows, :])
                sq = work.tile([P, D], fp32, name="sqz", tag="sq")
                nc.scalar.activation(sq[:], z_t[:], AF.Square,
                                     accum_out=zn2[:, tt:tt + 1])
                for dt in range(DT):
                    ztp = psum(f"ztp{tt}_{dt}")
                    nc.tensor.transpose(ztp[:, :P], z_t[:, dt * P:(dt + 1) * P], ident[:])
                    nc.vector.tensor_copy(zT[dt][:, tt * P:(tt + 1) * P], ztp[:, :P])
            nc.scalar.activation(zninv[:], zn2[:], AF.Sqrt, bias=eps_sb[:])
            nc.vector.reciprocal(zninv[:], zninv[:])

            # ---- Phase 3: dots, softmax, attn out, attn transpose ----
            attnT = [poolAt.tile([P, TT * P], fp32, name=f"attnT{i}") for i in range(PT)]
            for tt in range(TT):
                ps_d = psum(f"dots{tt}")
                for dt in range(DT):
                    nc.tensor.matmul(ps_d[:, :K], lhsT=zT[dt][:, tt * P:(tt + 1) * P],
                                     rhs=ptn[dt][:], start=(dt == 0), stop=(dt == DT - 1))
                at = poolAt.tile([P, K], fp32, name=f"attn{tt}")
                nc.scalar.activation(at[:], ps_d[:, :K], AF.Copy,
                                     scale=zninv[:, tt:tt + 1])
                nmax = work.tile([P, 1], fp32, name="nmax", tag="nmax")
                nc.vector.tensor_reduce(out=nmax[:], in_=at[:], axis=AX.X,
                                        op=ALU.max, negate=True)
                esum = work.tile([P, 1], fp32, name="esum", tag="esum")
                nc.scalar.activation(at[:], at[:], AF.Exp, bias=nmax[:],
                                     accum_out=esum[:])
                nc.vector.reciprocal(esum[:], esum[:])
                nc.scalar.activation(at[:], at[:], AF.Copy, scale=esum[:])
                rows = P if tt < 6 else T - 6 * P
                nc.sync.dma_start(attn_o[tt * P: tt * P + rows, :], at[:rows, :])
                for pt in range(PT):
                    atp = psum(f"atp{tt}_{pt}")
                    nc.tensor.transpose(atp[:, :P], at[:, pt * P:(pt + 1) * P], ident[:])
                    nc.vector.tensor_copy(attnT[pt][:, tt * P:(tt + 1) * P], atp[:, :P])

            # ---- Phase 4: z_hatT = (attn @ P)^T ----
            NSP = TT * P // 2  # 448
            zhT = [poolZ.tile([P, TT * P], fp32, name=f"zhT{i}") for i in range(DT)]
            for dt in range(DT):
                for n2 in range(2):
                    sl = slice(n2 * NSP, (n2 + 1) * NSP)
                    ps_z = psum(f"zh{dt}_{n2}")
                    for pt in range(PT):
                        nc.tensor.matmul(ps_z[:, :NSP],
                                         lhsT=p_nat[pt][:, dt * P:(dt + 1) * P],
                                         rhs=attnT[pt][:, sl],
                                         start=(pt == 0), stop=(pt == PT - 1))
                    nc.vector.tensor_copy(zhT[dt][:, sl], ps_z[:, :NSP])
            # ---- Phase 5: gate, fused (in-place into zhT), pooled ----
            stagePA.close()  # free prototype/attn pools before gate weights
            poolG = stageA.enter_context(tc.tile_pool(name="poolG", bufs=1))
            gw_sb = [poolG.tile([P, D], fp32, name=f"gw{i}") for i in range(16)]
            for kt in range(16):
                nc.sync.dma_start(gw_sb[kt][:], gw_d[kt * P:(kt + 1) * P, :])
            g_all = [poolZ.tile([P, TT * P], fp32, name=f"g_all{i}") for i in range(DT)]
            pooledT = const.tile([P, DT, BL], fp32, name="pooledT")
            # 5a: all gates first (zhT must stay pristine while it feeds rhs)
            for dt in range(DT):
                for n2 in range(2):
                    sl = slice(n2 * NSP, (n2 + 1) * NSP)
                    ps_g = psum(f"g{dt}_{n2}")
                    for kt in range(16):
                        rhs = zT[kt][:, sl] if kt < 8 else zhT[kt - 8][:, sl]
                        nc.tensor.matmul(ps_g[:, :NSP],
                                         lhsT=gw_sb[kt][:, dt * P:(dt + 1) * P],
                                         rhs=rhs, start=(kt == 0), stop=(kt == 15))
                    nc.scalar.activation(g_all[dt][:, sl], ps_g[:, :NSP], AF.Sigmoid,
                                         bias=gb_sb[:, dt:dt + 1])
            # 5b: fused (in-place into zhT) + pooling
            for dt in range(DT):
                for n2 in range(2):
                    sl = slice(n2 * NSP, (n2 + 1) * NSP)
                    dif = work.tile([P, NSP], fp32, name="dif", tag="dif")
                    nc.vector.tensor_sub(dif[:], zT[dt][:, sl], zhT[dt][:, sl])
                    nc.vector.tensor_mul(dif[:], g_all[dt][:, sl], dif[:])
                    nc.vector.tensor_add(zhT[dt][:, sl], zhT[dt][:, sl], dif[:])
                for s in range(BL):
                    nc.vector.reduce_sum(out=pooledT[:, dt, s:s + 1],
                                         in_=zhT[dt][:, s * N:(s + 1) * N], axis=AX.X)
            nc.vector.tensor_scalar_mul(pooledT[:], pooledT[:], 1.0 / N)

        # ---- Phase 6: AllToAll pooled ----
        a2a_in = dram.tile([D, BL], fp32, name="a2a_in")
        a2a_out = dram.tile([D, BL], fp32, name="a2a_out")
        nc.sync.dma_start(a2a_in[:].rearrange("(o p) s -> p o s", p=P), pooledT[:])
        nc.gpsimd.collective_compute("AllToAll", ALU.bypass, replica_groups=RG,
                                     ins=[a2a_in.opt()], outs=[a2a_out.opt()])
        pf = const.tile([P, NCORES, BL], fp32, name="pf")
        nc.sync.dma_start(pf[:], a2a_out[:].rearrange("(c p) s -> p c s", p=P))
        pf2 = pf[:].rearrange("p c s -> p (c s)")

        # ---- Phase 7: dec matmul (partial, my 128 input rows) + RS ----
        rs_in = dram.tile([B, DEC_N], fp32, name="rs_in")
        rs_out = dram.tile([BL, DEC_N], fp32, name="rs_out")
        with ExitStack() as stageD:
            poolD = stageD.enter_context(tc.tile_pool(name="poolD", bufs=3))
            for ch in range(DEC_CH):
                dw_t = poolD.tile([P, DEC_CW], fp32, name="dw_t", tag="dw_t")
                nc.sync.dma_start(dw_t[:], dw_d[:, ch * DEC_CW:(ch + 1) * DEC_CW])
                h_t = poolD.tile([B, DEC_CW], fp32, name="h_t", tag="h_t")
                for nn in range(DEC_CW // 512):
                    ps_h = psum(f"h{ch}_{nn}")
                    nc.tensor.matmul(ps_h[:B, :], lhsT=pf2,
                                     rhs=dw_t[:, nn * 512:(nn + 1) * 512],
                                     start=True, stop=True)
                    nc.vector.tensor_copy(h_t[:, nn * 512:(nn + 1) * 512], ps_h[:B, :])
                nc.sync.dma_start(rs_in[:, ch * DEC_CW:(ch + 1) * DEC_CW], h_t[:])
        nc.gpsimd.collective_compute("ReduceScatter", ALU.add, replica_groups=RG,
                                     ins=[rs_in.opt()], outs=[rs_out.opt()])

        # ---- Phase 8: convT1 (512->256, 7x7 -> 14x14) ----
        with ExitStack() as stageC:
            poolC = stageC.enter_context(tc.tile_pool(name="poolC", bufs=1))
            h4 = rs_out[:].rearrange("s (c h w) -> c s h w", c=512, h=7)
            xpad = [poolC.tile([P, BL, 9, 9], fp32, name=f"xpad{i}") for i in range(4)]
            w1 = [poolC.tile([P, 256, 4, 4], fp32, name=f"w1_{i}") for i in range(4)]
            db4 = db_d[:].rearrange("(c h w) -> c h w", c=512, h=7)
            for ci in range(4):
                nc.gpsimd.memset(xpad[ci][:], 0.0)
                for s in range(BL):
                    nc.sync.dma_start(xpad[ci][:, s, 1:8, 1:8],
                                      h4[ci * P:(ci + 1) * P, s])
                bd = work.tile([P, 7, 7], fp32, name="bd", tag="bd")
                nc.sync.dma_start(bd[:], db4[ci * P:(ci + 1) * P])
                nc.vector.tensor_add(xpad[ci][:, :, 1:8, 1:8], xpad[ci][:, :, 1:8, 1:8],
                                     bd[:][:, None].to_broadcast([P, BL, 7, 7]))
                nc.sync.dma_start(w1[ci][:], w1_d[ci * P:(ci + 1) * P])
            y1pad = [poolC.tile([P, BL, 16, 16], fp32, name=f"y1pad{i}") for i in range(2)]
            for mo in range(2):
                nc.gpsimd.memset(y1pad[mo][:], 0.0)
            for po in range(2):
                for pw in range(2):
                    for mo in range(2):
                        ps_c = psum(f"c1_{po}{pw}{mo}")
                        pc4 = ps_c[:, :196].rearrange("p (s h w) -> p s h w", s=BL, h=7)
                        idx = 0
                        for ci in range(4):
                            for (dh, kh) in TAPS[po]:
                                for (dw_, kw) in TAPS[pw]:
                                    nc.tensor.matmul(
                                        pc4,
                                        lhsT=w1[ci][:, mo * P:(mo + 1) * P, kh, kw],
                                        rhs=xpad[ci][:, :, dh:dh + 7, dw_:dw_ + 7],
                                        start=(idx == 0), stop=(idx == 15))
                                    idx += 1
                        nc.scalar.activation(
                            y1pad[mo][:, :, 1 + po:15:2, 1 + pw:15:2], pc4,
                            AF.Relu, bias=b1_sb[:, mo:mo + 1])

            # ---- Phase 9: convT2 (256->3, 14x14 -> 28x28) ----
            w2 = [poolC.tile([P, 3, 4, 4], fp32, name=f"w2_{i}") for i in range(2)]
            for ci in range(2):
                nc.sync.dma_start(w2[ci][:], w2_d[ci * P:(ci + 1) * P])
            xr = poolC.tile([3, BL, 28, 28], fp32, name="xr")
            for po in range(2):
                for pw in range(2):
                    for half in range(2):
                        ps_x = psum(f"c2_{po}{pw}{half}")
                        px4 = ps_x[:3, :392].rearrange("p (s h w) -> p s h w", s=BL, h=14)
                        idx = 0
                        for ci in range(2):
                            for (dh, kh) in TAPS[po]:
                                for (dw_, kw) in TAPS[pw]:
                                    nc.tensor.matmul(
                                        px4,
                                        lhsT=w2[ci][:, :, kh, kw],
                                        rhs=y1pad[ci][:, :, dh:dh + 14,
                                                      dw_ + 7 * half:dw_ + 7 * half + 7],
                                        start=(idx == 0), stop=(idx == 7))
                                    idx += 1
                        nc.scalar.activation(
                            xr[:, :, po:28:2, pw + 14 * half:pw + 14 * half + 13:2],
                            px4, AF.Identity, bias=b2_sb[:])
            nc.sync.dma_start(xr_o[:].rearrange("s c h w -> c s h w"), xr[:])
    _legalize_sync(nc)
    return nc


_NC_CACHE = None


def _get_nc():
    global _NC_CACHE
    if _NC_CACHE is None:
        _NC_CACHE = _build()
    return _NC_CACHE


def _run(inputs, trace=False):
    nc = _get_nc()
    z = np.ascontiguousarray(np.asarray(inputs["z"], dtype=np.float32))
    dec_w = np.asarray(inputs["dec_w"], dtype=np.float32)
    shared = {
        "prototypes": np.ascontiguousarray(inputs["prototypes"], dtype=np.float32),
        "gate_w": np.ascontiguousarray(inputs["gate_w"], dtype=np.float32),
        "gate_b": np.ascontiguousarray(inputs["gate_b"], dtype=np.float32),
        "dec_b": np.ascontiguousarray(inputs["dec_b"], dtype=np.float32),
        "ct1_w": np.ascontiguousarray(inputs["ct1_w"], dtype=np.float32),
        "ct1_b": np.ascontiguousarray(inputs["ct1_b"], dtype=np.float32),
        "ct2_w": np.ascontiguousarray(inputs["ct2_w"], dtype=np.float32),
        "ct2_b": np.ascontiguousarray(inputs["ct2_b"], dtype=np.float32),
    }
    in_maps = []
    for c in range(NCORES):
        m = dict(shared)
        m["z"] = np.ascontiguousarray(z[c * BL:(c + 1) * BL].reshape(T, D))
        m["dec_w_shard"] = np.ascontiguousarray(dec_w[c * P:(c + 1) * P])
        in_maps.append(m)
    res = run_bass_kernel_spmd(nc, in_maps, list(range(NCORES)), trace=trace)
    x_recon = np.concatenate([res.results[c]["xr_out"] for c in range(NCORES)], axis=0)
    attn = np.concatenate(
        [res.results[c]["attn_out"].reshape(BL, N, K) for c in range(NCORES)], axis=0)
    return (x_recon, attn), res.exec_time_ns


def kernel(**inputs):
    out, _ = _run(inputs, trace=False)
    return out


if __name__ == "__main__":
    rng = np.random.default_rng(0)
    ins = dict(
        z=rng.standard_normal((B, N, D), dtype=np.float32),
        prototypes=rng.standard_normal((K, D), dtype=np.float32),
        gate_w=(rng.standard_normal((2 * D, D), dtype=np.float32) * 0.02),
        gate_b=np.zeros(D, np.float32),
        dec_w=(rng.standard_normal((D, DEC_N), dtype=np.float32) * 0.02),
        dec_b=np.zeros(DEC_N, np.float32),
        ct1_w=(rng.standard_normal((512, 256, 4, 4), dtype=np.float32) * 0.02),
        ct1_b=np.zeros(256, np.float32),
        ct2_w=(rng.standard_normal((256, 3, 4, 4), dtype=np.float32) * 0.02),
        ct2_b=np.zeros(3, np.float32),
    )
    out, t = _run(ins, trace=False)
    print("shapes:", out[0].shape, out[1].shape, "exec_ns:", t)


# revision 15
# speedup vs baseline: 1.3129x; 1.2909x over previous
"""Trainium2 Bass kernel for MemVim (memory-bank attention + gated fusion + decoder).

Sharding: data-parallel over batch for the attention/gating stages (4 samples
per core), tensor-parallel (row-sharded dec_w) for the 1024x25088 decoder
linear with an AllToAll on pooled features and a ReduceScatter on the decoder
output, then data-parallel conv-transpose decode of each core's own samples.
"""
import sys, os, types

sys.path.insert(0, "/opt/trn_rl_repo")


def _install_ntff_hook():
    try:
        from antenv.axon_hooks import get_axon_ntff_profile_hook  # noqa
        return
    except ImportError:
        pass
    try:
        from trn_agent_boot.trn_boot import _ntff_profile_via_ctypes
        hook = _ntff_profile_via_ctypes("/opt/axon/libaxon_pjrt.so")
    except Exception:
        hook = None
    mod = types.ModuleType("antenv.axon_hooks")
    mod.get_axon_ntff_profile_hook = lambda: hook
    mod.set_axon_ntff_profile_hook = lambda h: None
    sys.modules["antenv.axon_hooks"] = mod


_install_ntff_hook()

import numpy as np
from contextlib import ExitStack
import concourse.bass as bass
import concourse.mybir as mybir
import concourse.tile as tile
from concourse.bass_utils import run_bass_kernel_spmd
from concourse.masks import make_identity
from concourse.vector_clock import ScopedClock, VectorClock


# The walrus build in this image rejects InstDrain carrying fused sync
# commands and instructions with >1 sync wait. Replace the stock barriers
# with sem-only (sequencer-level) barriers and spread the tail drain's
# global-clock waits across single-wait NOPs. The waits still prove every
# DMA/collective semaphore increment landed before the sem reset.
def _patched_meb(self, engines):
    for inst in self._sem_only_all_engine_barrier_insts("aeb"):
        self.engines[inst.engine].add_instruction(inst)


def _patched_dab(self, tick_clock, wait_clock):
    gc = tick_clock.global_clock
    n = len(gc)
    procs = [i for i in range(n) if gc[i] > 0]
    for j, p in enumerate(procs):
        nop_inst = self.nc.sync.nop(nofuse=True, hint=f"tailwait{j}")
        vec = [0] * n
        vec[p] = gc[p]
        wait_clock.add_sem_waits(nop_inst.ins, ScopedClock({None: VectorClock(vec)}))
    self.nc.sync.drain()
    self.nc.all_engine_barrier(sem_only=True)
    popped = self.nc._tile_sem_poison_stack.pop()
    assert popped is self._sem_poison
    self.nc.clear_and_free_semaphores(list(self.sems.allocated().values()))
    self.nc.all_engine_barrier(sem_only=True)


bass.Bass.multi_engine_barrier = _patched_meb
tile.TileContext._drain_and_barrier = _patched_dab

# No remote artifact bucket in this container; keep the profile local.
import concourse.bass_utils as _bu
_bu.upload_artifacts = lambda tmpdir: str(tmpdir)


def _legalize_sync(nc):
    """This walrus build encodes at most one sync wait and one sync update
    per instruction, and none at all on InstDrain. Move extra waits onto
    NOPs inserted just before the instruction (same engine => same stream
    order) and extra updates onto NOPs just after."""
    ctr = [0]

    def nop(engine, waits, updates):
        ctr[0] += 1
        n = mybir.InstNoOp(name=f"lsw-{ctr[0]}", ins=[], outs=[])
        n.engine = engine
        n.sync_info = mybir.SyncInfo(on_wait=list(waits), on_update=list(updates))
        return n

    for fn in nc.m.functions:
        for bb in fn.blocks:
            out = []
            for ins in bb.instructions:
                si = ins.sync_info
                if si is None:
                    out.append(ins)
                    continue
                waits = list(si.on_wait)
                updates = list(si.on_update)
                is_drain = type(ins).__name__ == "InstDrain"
                kw = 0 if is_drain else 1
                ku = 0 if is_drain else 1
                extra_w = waits[kw:]
                extra_u = updates[ku:]
                if not extra_w and not extra_u:
                    out.append(ins)
                    continue
                for w in extra_w:
                    out.append(nop(ins.engine, [w], []))
                ins.sync_info = mybir.SyncInfo(on_wait=waits[:kw], on_update=updates[:ku])
                out.append(ins)
                for u in extra_u:
                    out.append(nop(ins.engine, [], [u]))
            bb.instructions = out

fp32 = mybir.dt.float32
bf16 = mybir.dt.bfloat16
AF = mybir.ActivationFunctionType
ALU = mybir.AluOpType
AX = mybir.AxisListType

NCORES = 8
P = 128
B, N, D, K = 32, 196, 1024, 512
BL = B // NCORES          # 4 samples per core
T = BL * N                # 784 tokens per core
TT = 7                    # token tiles (6 full + 1 of 16)
DT = D // P               # 8
PT = K // P               # 4
DEC_N = 7 * 7 * 512       # 25088
DEC_CH = 7                # dec column chunks
DEC_CW = DEC_N // DEC_CH  # 3584
RG = [list(range(NCORES))]

# taps[parity] = list of (pad_offset_delta, kernel_index) for k=4,s=2,p=1
TAPS = {0: [(1, 1), (0, 3)], 1: [(2, 0), (1, 2)]}


def _build():
    nc = bass.Bass()
    z_d = nc.dram_tensor("z", [T, D], fp32, kind="ExternalInput")
    p_d = nc.dram_tensor("prototypes", [K, D], fp32, kind="ExternalInput")
    gw_d = nc.dram_tensor("gate_w", [2 * D, D], fp32, kind="ExternalInput")
    gb_d = nc.dram_tensor("gate_b", [D], fp32, kind="ExternalInput")
    dw_d = nc.dram_tensor("dec_w_shard", [P, DEC_N], fp32, kind="ExternalInput")
    db_d = nc.dram_tensor("dec_b", [DEC_N], fp32, kind="ExternalInput")
    w1_d = nc.dram_tensor("ct1_w", [512, 256, 4, 4], fp32, kind="ExternalInput")
    b1_d = nc.dram_tensor("ct1_b", [256], fp32, kind="ExternalInput")
    w2_d = nc.dram_tensor("ct2_w", [256, 3, 4, 4], fp32, kind="ExternalInput")
    b2_d = nc.dram_tensor("ct2_b", [3], fp32, kind="ExternalInput")
    attn_o = nc.dram_tensor("attn_out", [T, K], fp32, kind="ExternalOutput")
    xr_o = nc.dram_tensor("xr_out", [BL, 3, 28, 28], fp32, kind="ExternalOutput")

    with tile.TileContext(nc) as tc, ExitStack() as ctx:
        const = ctx.enter_context(tc.tile_pool(name="const", bufs=1))
        dram = ctx.enter_context(tc.tile_pool(name="dram", bufs=1, space="DRAM"))
        ps = ctx.enter_context(tc.tile_pool(name="ps", bufs=8, space="PSUM"))
        work = ctx.enter_context(tc.tile_pool(name="work", bufs=2))

        def psum(name):
            return ps.tile([P, 512], fp32, tag="ps", name=name)

        ident = const.tile([P, P], fp32, name="ident")
        make_identity(nc, ident[:])
        eps_sb = const.tile([P, 1], fp32, name="eps_sb")
        nc.gpsimd.memset(eps_sb[:], 1e-30)
        gb_sb = const.tile([P, DT], fp32, name="gb_sb")
        nc.sync.dma_start(gb_sb[:], gb_d[:].rearrange("(o p) -> p o", p=P))
        b1_sb = const.tile([P, 2], fp32, name="b1_sb")
        nc.sync.dma_start(b1_sb[:], b1_d[:].rearrange("(o p) -> p o", p=P))
        b2_sb = const.tile([3, 1], fp32, name="b2_sb")
        nc.sync.dma_start(b2_sb[:], b2_d[:][:, None])

        with ExitStack() as stageA:
            poolZ = stageA.enter_context(tc.tile_pool(name="poolZ", bufs=1))
            stagePA = ExitStack()
            poolP = stagePA.enter_context(tc.tile_pool(name="poolP", bufs=1))
            poolAt = stagePA.enter_context(tc.tile_pool(name="poolAt", bufs=1))

            # ---- Phase 1: prototypes: load, row-normalize, transpose ----
            p_nat = [poolP.tile([P, D], fp32, name=f"p_nat{i}") for i in range(PT)]
            ptn = [poolP.tile([P, K], fp32, name=f"ptn{i}") for i in range(DT)]
            pn2 = const.tile([P, PT], fp32, name="pn2")
            pninv = const.tile([P, PT], fp32, name="pninv")
            for pt in range(PT):
                nc.sync.dma_start(p_nat[pt][:], p_d[pt * P:(pt + 1) * P, :])
                sq = work.tile([P, D], fp32, name="sq", tag="sq")
                nc.scalar.activation(sq[:], p_nat[pt][:], AF.Square,
                                     accum_out=pn2[:, pt:pt + 1])
            nc.scalar.activation(pninv[:], pn2[:], AF.Sqrt, bias=eps_sb[:])
            nc.vector.reciprocal(pninv[:], pninv[:])
            for pt in range(PT):
                pnrm = work.tile([P, D], fp32, name="pnrm", tag="pnrm")
                nc.scalar.activation(pnrm[:], p_nat[pt][:], AF.Copy,
                                     scale=pninv[:, pt:pt + 1])
                for dt in range(DT):
                    ptp = psum(f"ptp{pt}_{dt}")
                    nc.tensor.transpose(ptp[:, :P], pnrm[:, dt * P:(dt + 1) * P], ident[:])
                    nc.vector.tensor_copy(ptn[dt][:, pt * P:(pt + 1) * P], ptp[:, :P])

            # ---- Phase 2: z: load, norms, transpose ----
            zT = [poolZ.tile([P, TT * P], fp32, name=f"zT{i}") for i in range(DT)]
            zn2 = const.tile([P, TT], fp32, name="zn2")
            zninv = const.tile([P, TT], fp32, name="zninv")
            for tt in range(TT):
                z_t = work.tile([P, D], fp32, name="z_t", tag="z_t")
                rows = P if tt < 6 else T - 6 * P
                if rows < P:
                    nc.gpsimd.memset(z_t[:], 0.0)
                nc.sync.dma_start(z_t[:rows, :], z_d[tt * P: tt * P + rows, :])
                sq = work.tile([P, D], fp32, name="sqz", tag="sq")
                nc.scalar.activation(sq[:], z_t[:], AF.Square,
                                     accum_out=zn2[:, tt:tt + 1])
                for dt in range(DT):
                    ztp = psum(f"ztp{tt}_{dt}")
                    nc.tensor.transpose(ztp[:, :P], z_t[:, dt * P:(dt + 1) * P], ident[:])
                    nc.vector.tensor_copy(zT[dt][:, tt * P:(tt + 1) * P], ztp[:, :P])
            nc.scalar.activation(zninv[:], zn2[:], AF.Sqrt, bias=eps_sb[:])
            nc.vector.reciprocal(zninv[:], zninv[:])

            # ---- Phase 3: dots, softmax, attn out, attn transpose ----
            attnT = [poolAt.tile([P, TT * P], fp32, name=f"attnT{i}") for i in range(PT)]
            for tt in range(TT):
                ps_d = psum(f"dots{tt}")
                for dt in range(DT):
                    nc.tensor.matmul(ps_d[:, :K], lhsT=zT[dt][:, tt * P:(tt + 1) * P],
                                     rhs=ptn[dt][:], start=(dt == 0), stop=(dt == DT - 1))
                at = poolAt.tile([P, K], fp32, name=f"attn{tt}")
                nc.scalar.activation(at[:], ps_d[:, :K], AF.Copy,
                                     scale=zninv[:, tt:tt + 1])
                nmax = work.tile([P, 1], fp32, name="nmax", tag="nmax")
                nc.vector.tensor_reduce(out=nmax[:], in_=at[:], axis=AX.X,
                                        op=ALU.max, negate=True)
                esum = work.tile([P, 1], fp32, name="esum", tag="esum")
                nc.scalar.activation(at[:], at[:], AF.Exp, bias=nmax[:],
                                     accum_out=esum[:])
                nc.vector.reciprocal(esum[:], esum[:])
                nc.scalar.activation(at[:], at[:], AF.Copy, scale=esum[:])
                rows = P if tt < 6 else T - 6 * P
                nc.sync.dma_start(attn_o[tt * P: tt * P + rows, :], at[:rows, :])
                for pt in range(PT):
                    atp = psum(f"atp{tt}_{pt}")
                    nc.tensor.transpose(atp[:, :P], at[:, pt * P:(pt + 1) * P], ident[:])
                    nc.vector.tensor_copy(attnT[pt][:, tt * P:(tt + 1) * P], atp[:, :P])

            # ---- Phase 4: z_hatT = (attn @ P)^T ----
            NSP = TT * P // 2  # 448
            zhT = [poolZ.tile([P, TT * P], fp32, name=f"zhT{i}") for i in range(DT)]
            for dt in range(DT):
                for n2 in range(2):
                    sl = slice(n2 * NSP, (n2 + 1) * NSP)
                    ps_z = psum(f"zh{dt}_{n2}")
                    for pt in range(PT):
                        nc.tensor.matmul(ps_z[:, :NSP],
                                         lhsT=p_nat[pt][:, dt * P:(dt + 1) * P],
                                         rhs=attnT[pt][:, sl],
                                         start=(pt == 0), stop=(pt == PT - 1))
                    nc.vector.tensor_copy(zhT[dt][:, sl], ps_z[:, :NSP])
            # ---- Phase 5: gate, fused (in-place into zhT), pooled ----
            stagePA.close()  # free prototype/attn pools before gate weights
            poolG = stageA.enter_context(tc.tile_pool(name="poolG", bufs=1))
            gwb = [poolG.tile([P, D], bf16, name=f"gwb{i}") for i in range(16)]
            for kt in range(16):
                gw_f = work.tile([P, D], fp32, name="gw_f", tag="sq")
                nc.sync.dma_start(gw_f[:], gw_d[kt * P:(kt + 1) * P, :])
                nc.vector.tensor_copy(gwb[kt][:], gw_f[:])
            zTb = [poolG.tile([P, TT * P], bf16, name=f"zTb{i}") for i in range(DT)]
            zhTb = [poolG.tile([P, TT * P], bf16, name=f"zhTb{i}") for i in range(DT)]
            for dt in range(DT):
                nc.vector.tensor_copy(zTb[dt][:], zT[dt][:])
                nc.vector.tensor_copy(zhTb[dt][:], zhT[dt][:])
            g_all = [poolZ.tile([P, TT * P], fp32, name=f"g_all{i}") for i in range(DT)]
            pooledT = const.tile([P, DT, BL], fp32, name="pooledT")
            # 5a: all gates first (zhT must stay pristine while it feeds rhs)
            for dt in range(DT):
                for n2 in range(2):
                    sl = slice(n2 * NSP, (n2 + 1) * NSP)
                    ps_g = psum(f"g{dt}_{n2}")
                    for kt in range(16):
                        rhs = zTb[kt][:, sl] if kt < 8 else zhTb[kt - 8][:, sl]
                        nc.tensor.matmul(ps_g[:, :NSP],
                                         lhsT=gwb[kt][:, dt * P:(dt + 1) * P],
                                         rhs=rhs, start=(kt == 0), stop=(kt == 15))
                    nc.scalar.activation(g_all[dt][:, sl], ps_g[:, :NSP], AF.Sigmoid,
                                         bias=gb_sb[:, dt:dt + 1])
            # 5b: fused (in-place into zhT) + pooling
            for dt in range(DT):
                for n2 in range(2):
                    sl = slice(n2 * NSP, (n2 + 1) * NSP)
                    dif = work.tile([P, NSP], fp32, name="dif", tag="dif")
                    nc.vector.tensor_sub(dif[:], zT[dt][:, sl], zhT[dt][:, sl])
                    nc.vector.tensor_mul(dif[:], g_all[dt][:, sl], dif[:])
                    nc.vector.tensor_add(zhT[dt][:, sl], zhT[dt][:, sl], dif[:])
                for s in range(BL):
                    nc.vector.reduce_sum(out=pooledT[:, dt, s:s + 1],
                                         in_=zhT[dt][:, s * N:(s + 1) * N], axis=AX.X)
            nc.vector.tensor_scalar_mul(pooledT[:], pooledT[:], 1.0 / N)

        # ---- Phase 6: AllToAll pooled ----
        a2a_in = dram.tile([D, BL], fp32, name="a2a_in")
        a2a_out = dram.tile([D, BL], fp32, name="a2a_out")
        nc.sync.dma_start(a2a_in[:].rearrange("(o p) s -> p o s", p=P), pooledT[:])
        nc.gpsimd.collective_compute("AllToAll", ALU.bypass, replica_groups=RG,
                                     ins=[a2a_in.opt()], outs=[a2a_out.opt()])
        pf = const.tile([P, NCORES, BL], fp32, name="pf")
        nc.sync.dma_start(pf[:], a2a_out[:].rearrange("(c p) s -> p c s", p=P))
        pf2 = pf[:].rearrange("p c s -> p (c s)")

        # ---- Phase 7: dec matmul (partial, my 128 input rows) + RS ----
        rs_in = dram.tile([B, DEC_N], fp32, name="rs_in")
        rs_out = dram.tile([BL, DEC_N], fp32, name="rs_out")
        with ExitStack() as stageD:
            poolD = stageD.enter_context(tc.tile_pool(name="poolD", bufs=3))
            for ch in range(DEC_CH):
                dw_t = poolD.tile([P, DEC_CW], fp32, name="dw_t", tag="dw_t")
                nc.sync.dma_start(dw_t[:], dw_d[:, ch * DEC_CW:(ch + 1) * DEC_CW])
                h_t = poolD.tile([B, DEC_CW], fp32, name="h_t", tag="h_t")
                for nn in range(DEC_CW // 512):
                    ps_h = psum(f"h{ch}_{nn}")
                    nc.tensor.matmul(ps_h[:B, :], lhsT=pf2,
                                     rhs=dw_t[:, nn * 512:(nn + 1) * 512],
                                     start=True, stop=True)
                    nc.vector.tensor_copy(h_t[:, nn * 512:(nn + 1) * 512], ps_h[:B, :])
                nc.sync.dma_start(rs_in[:, ch * DEC_CW:(ch + 1) * DEC_CW], h_t[:])
        nc.gpsimd.collective_compute("ReduceScatter", ALU.add, replica_groups=RG,
                                     ins=[rs_in.opt()], outs=[rs_out.opt()])

        # ---- Phase 8: convT1 (512->256, 7x7 -> 14x14) ----
        with ExitStack() as stageC:
            poolC = stageC.enter_context(tc.tile_pool(name="poolC", bufs=1))
            h4 = rs_out[:].rearrange("s (c h w) -> c s h w", c=512, h=7)
            xpad = [poolC.tile([P, BL, 9, 9], fp32, name=f"xpad{i}") for i in range(4)]
            w1 = [poolC.tile([P, 256, 4, 4], fp32, name=f"w1_{i}") for i in range(4)]
            db4 = db_d[:].rearrange("(c h w) -> c h w", c=512, h=7)
            for ci in range(4):
                nc.gpsimd.memset(xpad[ci][:], 0.0)
                for s in range(BL):
                    nc.sync.dma_start(xpad[ci][:, s, 1:8, 1:8],
                                      h4[ci * P:(ci + 1) * P, s])
                bd = work.tile([P, 7, 7], fp32, name="bd", tag="bd")
                nc.sync.dma_start(bd[:], db4[ci * P:(ci + 1) * P])
                nc.vector.tensor_add(xpad[ci][:, :, 1:8, 1:8], xpad[ci][:, :, 1:8, 1:8],
                                     bd[:][:, None].to_broadcast([P, BL, 7, 7]))
                nc.sync.dma_start(w1[ci][:], w1_d[ci * P:(ci + 1) * P])
            y1pad = [poolC.tile([P, BL, 16, 16], fp32, name=f"y1pad{i}") for i in range(2)]
            for mo in range(2):
                nc.gpsimd.memset(y1pad[mo][:], 0.0)
            for po in range(2):
                for pw in range(2):
                    for mo in range(2):
                        ps_c = psum(f"c1_{po}{pw}{mo}")
                        pc4 = ps_c[:, :196].rearrange("p (s h w) -> p s h w", s=BL, h=7)
                        idx = 0
                        for ci in range(4):
                            for (dh, kh) in TAPS[po]:
                                for (dw_, kw) in TAPS[pw]:
                                    nc.tensor.matmul(
                                        pc4,
                                        lhsT=w1[ci][:, mo * P:(mo + 1) * P, kh, kw],
                                        rhs=xpad[ci][:, :, dh:dh + 7, dw_:dw_ + 7],
                                        start=(idx == 0), stop=(idx == 15))
                                    idx += 1
                        nc.scalar.activation(
                            y1pad[mo][:, :, 1 + po:15:2, 1 + pw:15:2], pc4,
                            AF.Relu, bias=b1_sb[:, mo:mo + 1])

            # ---- Phase 9: convT2 (256->3, 14x14 -> 28x28) ----
            w2 = [poolC.tile([P, 3, 4, 4], fp32, name=f"w2_{i}") for i in range(2)]
            for ci in range(2):
                nc.sync.dma_start(w2[ci][:], w2_d[ci * P:(ci + 1) * P])
            xr = poolC.tile([3, BL, 28, 28], fp32, name="xr")
            for po in range(2):
                for pw in range(2):
                    for half in range(2):
                        ps_x = psum(f"c2_{po}{pw}{half}")
                        px4 = ps_x[:3, :392].rearrange("p (s h w) -> p s h w", s=BL, h=14)
                        idx = 0
                        for ci in range(2):
                            for (dh, kh) in TAPS[po]:
                                for (dw_, kw) in TAPS[pw]:
                                    nc.tensor.matmul(
                                        px4,
                                        lhsT=w2[ci][:, :, kh, kw],
                                        rhs=y1pad[ci][:, :, dh:dh + 14,
                                                      dw_ + 7 * half:dw_ + 7 * half + 7],
                                        start=(idx == 0), stop=(idx == 7))
                                    idx += 1
                        nc.scalar.activation(
                            xr[:, :, po:28:2, pw + 14 * half:pw + 14 * half + 13:2],
                            px4, AF.Identity, bias=b2_sb[:])
            nc.sync.dma_start(xr_o[:].rearrange("s c h w -> c s h w"), xr[:])
    _legalize_sync(nc)
    return nc


_NC_CACHE = None


def _get_nc():
    global _NC_CACHE
    if _NC_CACHE is None:
        _NC_CACHE = _build()
    return _NC_CACHE


def _run(inputs, trace=False):
    nc = _get_nc()
    z = np.ascontiguousarray(np.asarray(inputs["z"], dtype=np.float32))
    dec_w = np.asarray(inputs["dec_w"], dtype=np.float32)
    shared = {
        "prototypes": np.ascontiguousarray(inputs["prototypes"], dtype=np.float32),
        "gate_w": np.ascontiguousarray(inputs["gate_w"], dtype=np.float32),
        "gate_b": np.ascontiguousarray(inputs["gate_b"], dtype=np.float32),
        "dec_b": np.ascontiguousarray(inputs["dec_b"], dtype=np.float32),
        "ct1_w": np.ascontiguousarray(inputs["ct1_w"], dtype=np.float32),
        "ct1_b": np.ascontiguousarray(inputs["ct1_b"], dtype=np.float32),
        "ct2_w": np.ascontiguousarray(inputs["ct2_w"], dtype=np.float32),
        "ct2_b": np.ascontiguousarray(inputs["ct2_b"], dtype=np.float32),
    }
    in_maps = []
    for c in range(NCORES):
        m = dict(shared)
        m["z"] = np.ascontiguousarray(z[c * BL:(c + 1) * BL].reshape(T, D))
        m["dec_w_shard"] = np.ascontiguousarray(dec_w[c * P:(c + 1) * P])
        in_maps.append(m)
    res = run_bass_kernel_spmd(nc, in_maps, list(range(NCORES)), trace=trace)
    x_recon = np.concatenate([res.results[c]["xr_out"] for c in range(NCORES)], axis=0)
    attn = np.concatenate(
        [res.results[c]["attn_out"].reshape(BL, N, K) for c in range(NCORES)], axis=0)
    return (x_recon, attn), res.exec_time_ns


def kernel(**inputs):
    out, _ = _run(inputs, trace=False)
    return out


if __name__ == "__main__":
    rng = np.random.default_rng(0)
    ins = dict(
        z=rng.standard_normal((B, N, D), dtype=np.float32),
        prototypes=rng.standard_normal((K, D), dtype=np.float32),
        gate_w=(rng.standard_normal((2 * D, D), dtype=np.float32) * 0.02),
        gate_b=np.zeros(D, np.float32),
        dec_w=(rng.standard_normal((D, DEC_N), dtype=np.float32) * 0.02),
        dec_b=np.zeros(DEC_N, np.float32),
        ct1_w=(rng.standard_normal((512, 256, 4, 4), dtype=np.float32) * 0.02),
        ct1_b=np.zeros(256, np.float32),
        ct2_w=(rng.standard_normal((256, 3, 4, 4), dtype=np.float32) * 0.02),
        ct2_b=np.zeros(3, np.float32),
    )
    out, t = _run(ins, trace=False)
    print("shapes:", out[0].shape, out[1].shape, "exec_ns:", t)
